# revision 23
# baseline (speedup 1.0000x reference)
"""Trainium2 Bass kernel for supervised contrastive loss over N=8192 rows.

Strategy (8-core SPMD, rows sharded 1024/core):
  - Per column chunk t (128 cols): simT[c, r] = emb_cols_t @ emb_rows.T via PE,
    exp(sim/T) on the scalar engine (bf16 out), diagonal zeroed by an off-diag
    mask multiply, then S_T[class, row] += onehot_colsT @ exp on PE (bf16).
    Classes partition the columns, so total_sum = sum_c S_T and positive_sum =
    sum_c S_T * onehotR.  A small per-row-chunk matmul tail produces per-row
    masked loss; the host sums partials and divides by the valid count.
  - The diagonal's chunk position is made core-invariant by rotating each
    core's column-side data (embeddings and one-hots) by its row offset.
"""

import os
import numpy as np
import ml_dtypes

import concourse.tile as tile
from concourse import bacc, mybir
from concourse.bass_utils import run_bass_kernel_spmd

N, D, C = 8192, 128, 100
NCORES = 8
R = N // NCORES  # rows per core
NT = N // 128  # column chunks of 128
RC = R // 128  # row chunks per core (8)
TEMP = 0.07
F32 = mybir.dt.float32
F32R = mybir.dt.float32r
F16 = mybir.dt.float16
BF16 = mybir.dt.bfloat16

_PROGRAM_CACHE = {}


def _build_program(mm1_mode):
    mm1_dt = {"f16": F16, "bf16": BF16, "f32r": F32R, "f32": F32}[mm1_mode]
    emb_np_dt = {
        "f16": np.float16,
        "bf16": ml_dtypes.bfloat16,
        "f32r": np.float32,
        "f32": np.float32,
    }[mm1_mode]

    nc = bacc.Bacc("TRN2", target_bir_lowering=False, debug=False, num_devices=NCORES)

    emb_dram_dt = {"f16": F16, "bf16": BF16, "f32r": F32, "f32": F32}[mm1_mode]
    embT_cols = nc.dram_tensor("embT_cols", [D, N], emb_dram_dt, kind="ExternalInput")
    embT_rows = nc.dram_tensor("embT_rows", [D, R], emb_dram_dt, kind="ExternalInput")
    ohc = nc.dram_tensor("ohc", [N, C], BF16, kind="ExternalInput")
    ohrT = nc.dram_tensor("ohrT", [C, R], BF16, kind="ExternalInput")
    negval = nc.dram_tensor("negval", [128, RC], F32, kind="ExternalInput")
    offdiag = nc.dram_tensor("offdiag", [128, 128], BF16, kind="ExternalInput")
    out = nc.dram_tensor("out", [128, RC], F32, kind="ExternalOutput")

    def bc(ap):
        # view a DRAM fp32 AP as fp32r when needed so the verifier sees
        # fp32r-typed producers for fp32r matmuls
        return ap.bitcast(F32R) if mm1_mode == "f32r" else ap

    with tile.TileContext(nc) as tc:
        with (
            tc.tile_pool(name="consts", bufs=1) as consts,
            tc.tile_pool(name="spool", bufs=1, space="PSUM") as spool,
            tc.tile_pool(name="simpool", bufs=1, space="PSUM") as simpool,
            tc.tile_pool(name="exppool", bufs=2) as exppool,
            tc.tile_pool(name="fsb", bufs=1) as fsb,
        ):
            # Resident inputs, ordered so the first chunk's dependencies land
            # first: rows, cols[0], ohc[0], then the rest streams behind
            # compute.
            # Critical-path loads first, in small pieces, so chunk 0's matmul
            # and accumulation unblock within ~1us of kernel start.
            rows_sb = consts.tile([D, R], mm1_dt, tag="rows")
            cols_sb = []
            for j in range(8):
                tcol = consts.tile([D, 1024], mm1_dt, tag=f"col{j}", name=f"cols_sb{j}")
                cols_sb.append(tcol)
            ohc_sb = consts.tile([128, NT, C], BF16, tag="ohc")
            ohc_re = ohc[:, :].rearrange("(t p) c -> p t c", p=128)

            # Embeddings on the sync queue (critical path), one-hots and small
            # constants on the gpsimd queue in parallel.
            nc.sync.dma_start(cols_sb[0][:, 0:256], bc(embT_cols[:, 0:256]))
            nc.sync.dma_start(rows_sb[:, 0:512], bc(embT_rows[:, 0:512]))
            nc.sync.dma_start(rows_sb[:, 512:R], bc(embT_rows[:, 512:R]))
            nc.sync.dma_start(cols_sb[0][:, 256:1024], bc(embT_cols[:, 256:1024]))
            for j in range(1, 8):
                nc.sync.dma_start(
                    cols_sb[j][:], bc(embT_cols[:, j * 1024 : (j + 1) * 1024])
                )
            offd_sb = consts.tile([128, 128], BF16, tag="offd")
            nc.gpsimd.dma_start(ohc_sb[:, 0:2, :], ohc_re[:, 0:2, :])
            nc.gpsimd.dma_start(offd_sb[:], offdiag[:, :])
            nc.gpsimd.dma_start(ohc_sb[:, 2:8, :], ohc_re[:, 2:8, :])
            for j in range(1, 8):
                sl = slice(j * 8, (j + 1) * 8)
                nc.gpsimd.dma_start(ohc_sb[:, sl, :], ohc_re[:, sl, :])
            ohrT_sb = consts.tile([C, R], BF16, tag="ohrT")
            nc.gpsimd.dma_start(ohrT_sb[:], ohrT[:, :])
            nv_sb = consts.tile([128, RC], F32, tag="nv")
            nc.gpsimd.dma_start(nv_sb[:], negval[:, :])

            # Preload the Ln activation table while the pipeline ramps so the
            # tail's Ln doesn't pay the table switch.
            lnpre = fsb.tile([1, 1], F32, tag="lnpre")
            nc.vector.memset(lnpre[:], 1.0)
            lnpre_out = fsb.tile([1, 1], F32, tag="lnpre_out")
            nc.scalar.activation(
                lnpre_out[:], lnpre[:], mybir.ActivationFunctionType.Ln
            )

            # S_T[class, row] accumulator over all column chunks. Split into
            # two 512-row tiles: a matmul output must stay within one PSUM bank.
            S_T = [
                spool.tile([C, 512], F32, tag=f"S{q}", name=f"S_T{q}")
                for q in range(2)
            ]

            # Column chunks are processed in alternating groups of 2 and 1 so
            # one [128, 2048] (4-bank) and one [128, 1024] (2-bank) PSUM tile
            # ping-pong, amortizing the per-ACTIVATE fixed overhead over more
            # elements. Software-pipelined: group g+1's sim+exp are issued
            # before group g's accumulation matmuls.
            groups = []
            t = 0
            while t < NT:
                if len(groups) % 2 == 0 and t + 1 < NT:
                    groups.append((t, t + 1))
                    t += 2
                else:
                    groups.append((t,))
                    t += 1

            exp_of_group = [None] * len(groups)

            def emit_sim_exp(g):
                chunks = groups[g]
                n = len(chunks)
                tag = "simbig" if n == 2 else "simsmall"
                sim_ps = simpool.tile([128, n * R], F32, name=f"sim{g}", tag=tag)
                for i, tt in enumerate(chunks):
                    lhsT = cols_sb[tt // 8][:, (tt % 8) * 128 : (tt % 8 + 1) * 128]
                    for h in range(2):
                        osl = slice(i * R + h * 512, i * R + (h + 1) * 512)
                        rsl = slice(h * 512, (h + 1) * 512)
                        nc.tensor.matmul(
                            sim_ps[:, osl], lhsT, rows_sb[:, rsl], start=True, stop=True
                        )
                etag = "expbig" if n == 2 else "expsmall"
                exp_sb = exppool.tile([128, n * R], BF16, name=f"exp{g}", tag=etag)
                nc.scalar.activation(
                    exp_sb[:], sim_ps[:], mybir.ActivationFunctionType.Exp,
                    scale=float(1.0 / TEMP),
                )
                for i, tt in enumerate(chunks):
                    if tt < RC:
                        # Chunk tt's columns are rows tt*128..tt*128+127 of this
                        # core: the diagonal is the main diagonal of the block.
                        blk = slice(i * R + tt * 128, i * R + (tt + 1) * 128)
                        nc.vector.tensor_mul(
                            exp_sb[:, blk], exp_sb[:, blk], offd_sb[:]
                        )
                exp_of_group[g] = exp_sb

            emit_sim_exp(0)
            for g, chunks in enumerate(groups):
                if g + 1 < len(groups):
                    emit_sim_exp(g + 1)
                for i, tt in enumerate(chunks):
                    for q in range(2):
                        nc.tensor.matmul(
                            S_T[q][:],
                            ohc_sb[:, tt, :],
                            exp_of_group[g][:, i * R + q * 512 : i * R + (q + 1) * 512],
                            start=(tt == 0),
                            stop=(tt == NT - 1),
                        )

            # Tail: per-row totals via small matmuls so everything stays in a
            # [128, RC] layout (row = chunk*128 + partition).
            S_sb = fsb.tile([C, R], BF16, tag="S_sb")
            for q in range(2):
                nc.vector.tensor_copy(S_sb[:, q * 512 : (q + 1) * 512], S_T[q][:])
            posS = fsb.tile([C, R], BF16, tag="posS")
            nc.vector.tensor_mul(posS[:], S_sb[:], ohrT_sb[:])

            # Reuse the (released) sim PSUM slots for the tiny tail outputs —
            # all 8 banks are otherwise committed.
            tot_ps = simpool.tile([128, RC], F32, tag="simbig", name="tot_ps")
            pos_ps = simpool.tile([128, RC], F32, tag="simsmall", name="pos_ps")
            ones_bf = consts.tile([C, 1], BF16, tag="ones_bf")
            nc.vector.memset(ones_bf[:], 1.0)
            for j in range(RC):
                sl = slice(j * 128, (j + 1) * 128)
                nc.tensor.matmul(
                    tot_ps[:, j : j + 1], S_sb[:, sl], ones_bf[:], start=True, stop=True
                )
                nc.tensor.matmul(
                    pos_ps[:, j : j + 1], posS[:, sl], ones_bf[:], start=True, stop=True
                )
            t1 = fsb.tile([128, RC], F32, tag="t1")
            nc.vector.tensor_scalar_add(t1[:], tot_ps[:], 1e-8)
            rec = fsb.tile([128, RC], F32, tag="rec")
            nc.vector.reciprocal(rec[:], t1[:])
            ratio = fsb.tile([128, RC], F32, tag="ratio")
            nc.vector.tensor_mul(ratio[:], pos_ps[:], rec[:])
            nc.vector.tensor_scalar_add(ratio[:], ratio[:], 1e-8)
            lg = fsb.tile([128, RC], F32, tag="lg")
            nc.scalar.activation(lg[:], ratio[:], mybir.ActivationFunctionType.Ln)
            out_sb = fsb.tile([128, RC], F32, tag="out_sb")
            nc.vector.tensor_mul(out_sb[:], lg[:], nv_sb[:])
            nc.sync.dma_start(out[:, :], out_sb[:])

    nc.compile()
    return nc, emb_np_dt


def _get_program():
    mm1_mode = os.environ.get("CONTRASTIVE_MM1_DT", "f16")
    if mm1_mode not in _PROGRAM_CACHE:
        _PROGRAM_CACHE[mm1_mode] = _build_program(mm1_mode)
    return _PROGRAM_CACHE[mm1_mode]


def _prepare_in_maps(embeddings, labels, emb_np_dt):
    emb = np.asarray(embeddings, dtype=np.float32)
    lab = np.asarray(labels).astype(np.int64)
    embT = np.ascontiguousarray(emb.T).astype(emb_np_dt)  # [D, N]
    classes = np.arange(C, dtype=np.int64)
    onehot = lab[:, None] == classes[None, :]  # [N, C] bool
    oh_bf16 = onehot.astype(ml_dtypes.bfloat16)
    counts = np.bincount(lab, minlength=C)
    valid = (counts[lab] - 1) > 0  # [N] bool
    negval = np.where(valid, -1.0, 0.0).astype(np.float32)
    offd = (1.0 - np.eye(128, dtype=np.float32)).astype(ml_dtypes.bfloat16)

    in_maps = []
    for i in range(NCORES):
        r0 = i * R
        in_maps.append(
            {
                "embT_cols": np.ascontiguousarray(np.roll(embT, -r0, axis=1)),
                "embT_rows": np.ascontiguousarray(embT[:, r0 : r0 + R]),
                "ohc": np.ascontiguousarray(np.roll(oh_bf16, -r0, axis=0)),
                "ohrT": np.ascontiguousarray(oh_bf16[r0 : r0 + R].T),
                # [128, RC] with row = chunk*128 + partition
                "negval": np.ascontiguousarray(negval[r0 : r0 + R].reshape(RC, 128).T),
                "offdiag": offd,
            }
        )
    return in_maps, valid


def run(embeddings, labels, trace=False, trace_cores=None):
    """Returns (mean_loss, BassKernelResults)."""
    nc, emb_np_dt = _get_program()
    in_maps, valid = _prepare_in_maps(embeddings, labels, emb_np_dt)
    kwargs = {}
    if trace:
        kwargs["trace"] = True
        if trace_cores is not None:
            kwargs["trace_cores"] = trace_cores
    res = run_bass_kernel_spmd(nc, in_maps, core_ids=list(range(NCORES)), **kwargs)
    loss_sum = 0.0
    for i in range(NCORES):
        loss_sum += float(res.results[i]["out"].astype(np.float64).sum())
    cnt = int(valid.sum())
    mean = loss_sum / cnt if cnt > 0 else 0.0
    return np.asarray(mean, dtype=np.float32), res


def kernel(embeddings, labels):
    return run(embeddings, labels)[0]


# revision 24
# speedup vs baseline: 1.1791x; 1.1791x over previous
"""Trainium2 Bass kernel for supervised contrastive loss over N=8192 rows.

Strategy (8-core SPMD, rows sharded 1024/core):
  - Per column chunk t (128 cols): simT[c, r] = emb_cols_t @ emb_rows.T via PE,
    exp(sim/T) on the scalar engine (bf16 out), diagonal zeroed by an off-diag
    mask multiply, then S_T[class, row] += onehot_colsT @ exp on PE (bf16).
    Classes partition the columns, so total_sum = sum_c S_T and positive_sum =
    sum_c S_T * onehotR.  A small per-row-chunk matmul tail produces per-row
    masked loss; the host sums partials and divides by the valid count.
  - The diagonal's chunk position is made core-invariant by rotating each
    core's column-side data (embeddings and one-hots) by its row offset.
"""

import os
import numpy as np
import ml_dtypes

import concourse.tile as tile
from concourse import bacc, mybir
from concourse.bass_utils import run_bass_kernel_spmd

N, D, C = 8192, 128, 100
NCORES = 8
R = N // NCORES  # rows per core
NT = N // 128  # column chunks of 128
RC = R // 128  # row chunks per core (8)
TEMP = 0.07
F32 = mybir.dt.float32
F32R = mybir.dt.float32r
F16 = mybir.dt.float16
BF16 = mybir.dt.bfloat16

_PROGRAM_CACHE = {}


def _build_program(mm1_mode):
    mm1_dt = {"f16": F16, "bf16": BF16, "f32r": F32R, "f32": F32}[mm1_mode]
    emb_np_dt = {
        "f16": np.float16,
        "bf16": ml_dtypes.bfloat16,
        "f32r": np.float32,
        "f32": np.float32,
    }[mm1_mode]

    nc = bacc.Bacc("TRN2", target_bir_lowering=False, debug=False, num_devices=NCORES)

    emb_dram_dt = {"f16": F16, "bf16": BF16, "f32r": F32, "f32": F32}[mm1_mode]
    embT_cols = nc.dram_tensor("embT_cols", [D, N], emb_dram_dt, kind="ExternalInput")
    embT_rows = nc.dram_tensor("embT_rows", [D, R], emb_dram_dt, kind="ExternalInput")
    ohc = nc.dram_tensor("ohc", [N, C], BF16, kind="ExternalInput")
    ohrT = nc.dram_tensor("ohrT", [C, R], BF16, kind="ExternalInput")
    negval = nc.dram_tensor("negval", [128, RC], F32, kind="ExternalInput")
    offdiag = nc.dram_tensor("offdiag", [128, 128], BF16, kind="ExternalInput")
    out = nc.dram_tensor("out", [128, RC], F32, kind="ExternalOutput")

    def bc(ap):
        # view a DRAM fp32 AP as fp32r when needed so the verifier sees
        # fp32r-typed producers for fp32r matmuls
        return ap.bitcast(F32R) if mm1_mode == "f32r" else ap

    with tile.TileContext(nc) as tc:
        with (
            tc.tile_pool(name="consts", bufs=1) as consts,
            tc.tile_pool(name="spool", bufs=1, space="PSUM") as spool,
            tc.tile_pool(name="simpool", bufs=1, space="PSUM") as simpool,
            tc.tile_pool(name="exppool", bufs=2) as exppool,
            tc.tile_pool(name="fsb", bufs=1) as fsb,
        ):
            # Resident inputs, ordered so the first chunk's dependencies land
            # first: rows, cols[0], ohc[0], then the rest streams behind
            # compute.
            # Critical-path loads first, in small pieces, so chunk 0's matmul
            # and accumulation unblock within ~1us of kernel start.
            rows_sb = consts.tile([D, R], mm1_dt, tag="rows")
            cols_sb = []
            for j in range(8):
                tcol = consts.tile([D, 1024], mm1_dt, tag=f"col{j}", name=f"cols_sb{j}")
                cols_sb.append(tcol)
            ohc_sb = consts.tile([128, NT, C], BF16, tag="ohc")
            ohc_re = ohc[:, :].rearrange("(t p) c -> p t c", p=128)

            # Embeddings on the sync queue (critical path), one-hots and small
            # constants on the gpsimd queue in parallel.
            nc.sync.dma_start(cols_sb[0][:, 0:256], bc(embT_cols[:, 0:256]))
            nc.sync.dma_start(rows_sb[:, 0:512], bc(embT_rows[:, 0:512]))
            nc.sync.dma_start(rows_sb[:, 512:R], bc(embT_rows[:, 512:R]))
            nc.sync.dma_start(cols_sb[0][:, 256:1024], bc(embT_cols[:, 256:1024]))
            for j in range(1, 8):
                nc.sync.dma_start(
                    cols_sb[j][:], bc(embT_cols[:, j * 1024 : (j + 1) * 1024])
                )
            offd_sb = consts.tile([128, 128], BF16, tag="offd")
            nc.gpsimd.dma_start(ohc_sb[:, 0:2, :], ohc_re[:, 0:2, :])
            nc.gpsimd.dma_start(offd_sb[:], offdiag[:, :])
            nc.gpsimd.dma_start(ohc_sb[:, 2:8, :], ohc_re[:, 2:8, :])
            for j in range(1, 8):
                sl = slice(j * 8, (j + 1) * 8)
                nc.gpsimd.dma_start(ohc_sb[:, sl, :], ohc_re[:, sl, :])
            ohrT_sb = consts.tile([C, R], BF16, tag="ohrT")
            nc.gpsimd.dma_start(ohrT_sb[:], ohrT[:, :])
            nv_sb = consts.tile([128, RC], F32, tag="nv")
            nc.gpsimd.dma_start(nv_sb[:], negval[:, :])

            # Preload the Ln activation table while the pipeline ramps so the
            # tail's Ln doesn't pay the table switch.
            lnpre = fsb.tile([1, 1], F32, tag="lnpre")
            nc.vector.memset(lnpre[:], 1.0)
            lnpre_out = fsb.tile([1, 1], F32, tag="lnpre_out")
            nc.scalar.activation(
                lnpre_out[:], lnpre[:], mybir.ActivationFunctionType.Ln
            )

            # S_T[class, row] accumulator over all column chunks. Split into
            # two 512-row tiles: a matmul output must stay within one PSUM bank.
            S_T = [
                spool.tile([C, 512], F32, tag=f"S{q}", name=f"S_T{q}")
                for q in range(2)
            ]

            # Column chunks are processed in alternating groups of 2 and 1 so
            # one [128, 2048] (4-bank) and one [128, 1024] (2-bank) PSUM tile
            # ping-pong, amortizing the per-ACTIVATE fixed overhead over more
            # elements. Software-pipelined: group g+1's sim+exp are issued
            # before group g's accumulation matmuls.
            groups = []
            t = 0
            while t < NT:
                if len(groups) % 2 == 0 and t + 1 < NT:
                    groups.append((t, t + 1))
                    t += 2
                else:
                    groups.append((t,))
                    t += 1

            exp_of_group = [None] * len(groups)

            def emit_sim_exp(g):
                chunks = groups[g]
                n = len(chunks)
                tag = "simbig" if n == 2 else "simsmall"
                sim_ps = simpool.tile([128, n * R], F32, name=f"sim{g}", tag=tag)
                for i, tt in enumerate(chunks):
                    lhsT = cols_sb[tt // 8][:, (tt % 8) * 128 : (tt % 8 + 1) * 128]
                    for h in range(2):
                        osl = slice(i * R + h * 512, i * R + (h + 1) * 512)
                        rsl = slice(h * 512, (h + 1) * 512)
                        nc.tensor.matmul(
                            sim_ps[:, osl], lhsT, rows_sb[:, rsl], start=True, stop=True
                        )
                etag = "expbig" if n == 2 else "expsmall"
                exp_sb = exppool.tile([128, n * R], BF16, name=f"exp{g}", tag=etag)
                nc.scalar.activation(
                    exp_sb[:], sim_ps[:], mybir.ActivationFunctionType.Exp,
                    scale=float(1.0 / TEMP),
                )
                for i, tt in enumerate(chunks):
                    if tt < RC:
                        # Chunk tt's columns are rows tt*128..tt*128+127 of this
                        # core: the diagonal is the main diagonal of the block.
                        blk = slice(i * R + tt * 128, i * R + (tt + 1) * 128)
                        nc.vector.tensor_mul(
                            exp_sb[:, blk], exp_sb[:, blk], offd_sb[:]
                        )
                exp_of_group[g] = exp_sb

            emit_sim_exp(0)
            emit_sim_exp(1)
            for g, chunks in enumerate(groups):
                if g + 2 < len(groups):
                    emit_sim_exp(g + 2)
                for i, tt in enumerate(chunks):
                    for q in range(2):
                        nc.tensor.matmul(
                            S_T[q][:],
                            ohc_sb[:, tt, :],
                            exp_of_group[g][:, i * R + q * 512 : i * R + (q + 1) * 512],
                            start=(tt == 0),
                            stop=(tt == NT - 1),
                        )

            # Tail: per-row totals via small matmuls so everything stays in a
            # [128, RC] layout (row = chunk*128 + partition).
            S_sb = fsb.tile([C, R], BF16, tag="S_sb")
            for q in range(2):
                nc.vector.tensor_copy(S_sb[:, q * 512 : (q + 1) * 512], S_T[q][:])
            posS = fsb.tile([C, R], BF16, tag="posS")
            nc.vector.tensor_mul(posS[:], S_sb[:], ohrT_sb[:])

            # Reuse the (released) sim PSUM slots for the tiny tail outputs —
            # all 8 banks are otherwise committed.
            tot_ps = simpool.tile([128, RC], F32, tag="simbig", name="tot_ps")
            pos_ps = simpool.tile([128, RC], F32, tag="simsmall", name="pos_ps")
            ones_bf = consts.tile([C, 1], BF16, tag="ones_bf")
            nc.vector.memset(ones_bf[:], 1.0)
            for j in range(RC):
                sl = slice(j * 128, (j + 1) * 128)
                nc.tensor.matmul(
                    tot_ps[:, j : j + 1], S_sb[:, sl], ones_bf[:], start=True, stop=True
                )
                nc.tensor.matmul(
                    pos_ps[:, j : j + 1], posS[:, sl], ones_bf[:], start=True, stop=True
                )
            t1 = fsb.tile([128, RC], F32, tag="t1")
            nc.vector.tensor_scalar_add(t1[:], tot_ps[:], 1e-8)
            rec = fsb.tile([128, RC], F32, tag="rec")
            nc.vector.reciprocal(rec[:], t1[:])
            ratio = fsb.tile([128, RC], F32, tag="ratio")
            nc.vector.tensor_mul(ratio[:], pos_ps[:], rec[:])
            nc.vector.tensor_scalar_add(ratio[:], ratio[:], 1e-8)
            lg = fsb.tile([128, RC], F32, tag="lg")
            nc.scalar.activation(lg[:], ratio[:], mybir.ActivationFunctionType.Ln)
            out_sb = fsb.tile([128, RC], F32, tag="out_sb")
            nc.vector.tensor_mul(out_sb[:], lg[:], nv_sb[:])
            nc.sync.dma_start(out[:, :], out_sb[:])

    nc.compile()
    return nc, emb_np_dt


def _get_program():
    mm1_mode = os.environ.get("CONTRASTIVE_MM1_DT", "f16")
    if mm1_mode not in _PROGRAM_CACHE:
        _PROGRAM_CACHE[mm1_mode] = _build_program(mm1_mode)
    return _PROGRAM_CACHE[mm1_mode]


def _prepare_in_maps(embeddings, labels, emb_np_dt):
    emb = np.asarray(embeddings, dtype=np.float32)
    lab = np.asarray(labels).astype(np.int64)
    embT = np.ascontiguousarray(emb.T).astype(emb_np_dt)  # [D, N]
    classes = np.arange(C, dtype=np.int64)
    onehot = lab[:, None] == classes[None, :]  # [N, C] bool
    oh_bf16 = onehot.astype(ml_dtypes.bfloat16)
    counts = np.bincount(lab, minlength=C)
    valid = (counts[lab] - 1) > 0  # [N] bool
    negval = np.where(valid, -1.0, 0.0).astype(np.float32)
    offd = (1.0 - np.eye(128, dtype=np.float32)).astype(ml_dtypes.bfloat16)

    in_maps = []
    for i in range(NCORES):
        r0 = i * R
        in_maps.append(
            {
                "embT_cols": np.ascontiguousarray(np.roll(embT, -r0, axis=1)),
                "embT_rows": np.ascontiguousarray(embT[:, r0 : r0 + R]),
                "ohc": np.ascontiguousarray(np.roll(oh_bf16, -r0, axis=0)),
                "ohrT": np.ascontiguousarray(oh_bf16[r0 : r0 + R].T),
                # [128, RC] with row = chunk*128 + partition
                "negval": np.ascontiguousarray(negval[r0 : r0 + R].reshape(RC, 128).T),
                "offdiag": offd,
            }
        )
    return in_maps, valid


def run(embeddings, labels, trace=False, trace_cores=None):
    """Returns (mean_loss, BassKernelResults)."""
    nc, emb_np_dt = _get_program()
    in_maps, valid = _prepare_in_maps(embeddings, labels, emb_np_dt)
    kwargs = {}
    if trace:
        kwargs["trace"] = True
        if trace_cores is not None:
            kwargs["trace_cores"] = trace_cores
    res = run_bass_kernel_spmd(nc, in_maps, core_ids=list(range(NCORES)), **kwargs)
    loss_sum = 0.0
    for i in range(NCORES):
        loss_sum += float(res.results[i]["out"].astype(np.float64).sum())
    cnt = int(valid.sum())
    mean = loss_sum / cnt if cnt > 0 else 0.0
    return np.asarray(mean, dtype=np.float32), res


def kernel(embeddings, labels):
    return run(embeddings, labels)[0]


# revision 25
# speedup vs baseline: 1.1841x; 1.0042x over previous
"""Trainium2 Bass kernel for supervised contrastive loss over N=8192 rows.

Strategy (8-core SPMD, rows sharded 1024/core):
  - Per column chunk t (128 cols): simT[c, r] = emb_cols_t @ emb_rows.T via PE,
    exp(sim/T) on the scalar engine (bf16 out), diagonal zeroed by an off-diag
    mask multiply, then S_T[class, row] += onehot_colsT @ exp on PE (bf16).
    Classes partition the columns, so total_sum = sum_c S_T and positive_sum =
    sum_c S_T * onehotR.  A small per-row-chunk matmul tail produces per-row
    masked loss; the host sums partials and divides by the valid count.
  - The diagonal's chunk position is made core-invariant by rotating each
    core's column-side data (embeddings and one-hots) by its row offset.
"""

import os
import numpy as np
import ml_dtypes

import concourse.tile as tile
from concourse import bacc, mybir
from concourse.bass_utils import run_bass_kernel_spmd

N, D, C = 8192, 128, 100
NCORES = 8
R = N // NCORES  # rows per core
NT = N // 128  # column chunks of 128
RC = R // 128  # row chunks per core (8)
TEMP = 0.07
F32 = mybir.dt.float32
F32R = mybir.dt.float32r
F16 = mybir.dt.float16
BF16 = mybir.dt.bfloat16

_PROGRAM_CACHE = {}


def _build_program(mm1_mode):
    mm1_dt = {"f16": F16, "bf16": BF16, "f32r": F32R, "f32": F32}[mm1_mode]
    emb_np_dt = {
        "f16": np.float16,
        "bf16": ml_dtypes.bfloat16,
        "f32r": np.float32,
        "f32": np.float32,
    }[mm1_mode]

    nc = bacc.Bacc("TRN2", target_bir_lowering=False, debug=False, num_devices=NCORES)

    emb_dram_dt = {"f16": F16, "bf16": BF16, "f32r": F32, "f32": F32}[mm1_mode]
    embT_cols = nc.dram_tensor("embT_cols", [D, N], emb_dram_dt, kind="ExternalInput")
    embT_rows = nc.dram_tensor("embT_rows", [D, R], emb_dram_dt, kind="ExternalInput")
    ohc = nc.dram_tensor("ohc", [N, C], BF16, kind="ExternalInput")
    ohrT = nc.dram_tensor("ohrT", [C, R], BF16, kind="ExternalInput")
    negval = nc.dram_tensor("negval", [128, RC], F32, kind="ExternalInput")
    offdiag = nc.dram_tensor("offdiag", [128, 128], BF16, kind="ExternalInput")
    out = nc.dram_tensor("out", [128, RC], F32, kind="ExternalOutput")

    def bc(ap):
        # view a DRAM fp32 AP as fp32r when needed so the verifier sees
        # fp32r-typed producers for fp32r matmuls
        return ap.bitcast(F32R) if mm1_mode == "f32r" else ap

    with tile.TileContext(nc) as tc:
        with (
            tc.tile_pool(name="consts", bufs=1) as consts,
            tc.tile_pool(name="spool", bufs=1, space="PSUM") as spool,
            tc.tile_pool(name="simpool", bufs=1, space="PSUM") as simpool,
            tc.tile_pool(name="exppool", bufs=2) as exppool,
            tc.tile_pool(name="fsb", bufs=1) as fsb,
        ):
            # Resident inputs, ordered so the first chunk's dependencies land
            # first: rows, cols[0], ohc[0], then the rest streams behind
            # compute.
            # Critical-path loads first, in small pieces, so chunk 0's matmul
            # and accumulation unblock within ~1us of kernel start.
            rows_sb = consts.tile([D, R], mm1_dt, tag="rows")
            cols_sb = []
            for j in range(8):
                tcol = consts.tile([D, 1024], mm1_dt, tag=f"col{j}", name=f"cols_sb{j}")
                cols_sb.append(tcol)
            ohc_sb = consts.tile([128, NT, C], BF16, tag="ohc")
            ohc_re = ohc[:, :].rearrange("(t p) c -> p t c", p=128)

            # Embeddings on the sync queue (critical path), one-hots and small
            # constants on the gpsimd queue in parallel.
            nc.sync.dma_start(cols_sb[0][:, 0:256], bc(embT_cols[:, 0:256]))
            nc.sync.dma_start(rows_sb[:, 0:512], bc(embT_rows[:, 0:512]))
            nc.sync.dma_start(rows_sb[:, 512:R], bc(embT_rows[:, 512:R]))
            nc.sync.dma_start(cols_sb[0][:, 256:1024], bc(embT_cols[:, 256:1024]))
            for j in range(1, 8):
                nc.sync.dma_start(
                    cols_sb[j][:], bc(embT_cols[:, j * 1024 : (j + 1) * 1024])
                )
            offd_sb = consts.tile([128, 128], BF16, tag="offd")
            nc.gpsimd.dma_start(ohc_sb[:, 0:2, :], ohc_re[:, 0:2, :])
            nc.gpsimd.dma_start(offd_sb[:], offdiag[:, :])
            nc.gpsimd.dma_start(ohc_sb[:, 2:8, :], ohc_re[:, 2:8, :])
            for j in range(1, 8):
                sl = slice(j * 8, (j + 1) * 8)
                nc.gpsimd.dma_start(ohc_sb[:, sl, :], ohc_re[:, sl, :])
            ohrT_sb = consts.tile([C, R], BF16, tag="ohrT")
            nc.gpsimd.dma_start(ohrT_sb[:], ohrT[:, :])
            nv_sb = consts.tile([128, RC], F32, tag="nv")
            nc.gpsimd.dma_start(nv_sb[:], negval[:, :])

            # Preload the Ln activation table while the pipeline ramps so the
            # tail's Ln doesn't pay the table switch.
            lnpre = fsb.tile([1, 1], F32, tag="lnpre")
            nc.vector.memset(lnpre[:], 1.0)
            lnpre_out = fsb.tile([1, 1], F32, tag="lnpre_out")
            nc.scalar.activation(
                lnpre_out[:], lnpre[:], mybir.ActivationFunctionType.Ln
            )

            # S_T[class, row] accumulator over all column chunks. Split into
            # two 512-row tiles: a matmul output must stay within one PSUM bank.
            S_T = [
                spool.tile([C, 512], F32, tag=f"S{q}", name=f"S_T{q}")
                for q in range(2)
            ]

            # Column chunks are processed in alternating groups of 2 and 1 so
            # one [128, 2048] (4-bank) and one [128, 1024] (2-bank) PSUM tile
            # ping-pong, amortizing the per-ACTIVATE fixed overhead over more
            # elements. Software-pipelined: group g+1's sim+exp are issued
            # before group g's accumulation matmuls.
            groups = []
            t = 0
            while t < NT:
                if len(groups) % 2 == 0 and t + 1 < NT:
                    groups.append((t, t + 1))
                    t += 2
                else:
                    groups.append((t,))
                    t += 1

            exp_of_group = [None] * len(groups)

            def emit_sim_exp(g):
                chunks = groups[g]
                n = len(chunks)
                tag = "simbig" if n == 2 else "simsmall"
                sim_ps = simpool.tile([128, n * R], F32, name=f"sim{g}", tag=tag)
                for i, tt in enumerate(chunks):
                    lhsT = cols_sb[tt // 8][:, (tt % 8) * 128 : (tt % 8 + 1) * 128]
                    for h in range(2):
                        osl = slice(i * R + h * 512, i * R + (h + 1) * 512)
                        rsl = slice(h * 512, (h + 1) * 512)
                        nc.tensor.matmul(
                            sim_ps[:, osl], lhsT, rows_sb[:, rsl], start=True, stop=True
                        )
                etag = "expbig" if n == 2 else "expsmall"
                exp_sb = exppool.tile([128, n * R], BF16, name=f"exp{g}", tag=etag)
                nc.scalar.activation(
                    exp_sb[:], sim_ps[:], mybir.ActivationFunctionType.Exp,
                    scale=float(1.0 / TEMP),
                )
                for i, tt in enumerate(chunks):
                    if tt < RC:
                        # Chunk tt's columns are rows tt*128..tt*128+127 of this
                        # core: the diagonal is the main diagonal of the block.
                        blk = slice(i * R + tt * 128, i * R + (tt + 1) * 128)
                        nc.vector.tensor_mul(
                            exp_sb[:, blk], exp_sb[:, blk], offd_sb[:]
                        )
                exp_of_group[g] = exp_sb

            emit_sim_exp(0)
            emit_sim_exp(1)
            for g, chunks in enumerate(groups):
                if g + 2 < len(groups):
                    emit_sim_exp(g + 2)
                for i, tt in enumerate(chunks):
                    for q in range(2):
                        nc.tensor.matmul(
                            S_T[q][:],
                            ohc_sb[:, tt, :],
                            exp_of_group[g][:, i * R + q * 512 : i * R + (q + 1) * 512],
                            start=(tt == 0),
                            stop=(tt == NT - 1),
                        )

            # Tail: per-row totals via small matmuls so everything stays in a
            # [128, RC] layout (row = chunk*128 + partition).
            S_sb = fsb.tile([C, R], BF16, tag="S_sb")
            for q in range(2):
                nc.vector.tensor_copy(S_sb[:, q * 512 : (q + 1) * 512], S_T[q][:])
            posS = fsb.tile([C, R], BF16, tag="posS")
            nc.vector.tensor_mul(posS[:], S_sb[:], ohrT_sb[:])

            # Reuse the (released) sim PSUM slots for the tiny tail outputs —
            # all 8 banks are otherwise committed.
            tot_ps = simpool.tile([128, RC], F32, tag="simbig", name="tot_ps")
            pos_ps = simpool.tile([128, RC], F32, tag="simsmall", name="pos_ps")
            ones_bf = consts.tile([C, 1], BF16, tag="ones_bf")
            nc.vector.memset(ones_bf[:], 1.0)
            for j in range(RC):
                sl = slice(j * 128, (j + 1) * 128)
                nc.tensor.matmul(
                    tot_ps[:, j : j + 1], S_sb[:, sl], ones_bf[:], start=True, stop=True
                )
                nc.tensor.matmul(
                    pos_ps[:, j : j + 1], posS[:, sl], ones_bf[:], start=True, stop=True
                )
            t1 = fsb.tile([128, RC], F32, tag="t1")
            nc.vector.tensor_scalar_add(t1[:], tot_ps[:], 1e-8)
            rec = fsb.tile([128, RC], F32, tag="rec")
            nc.vector.reciprocal(rec[:], t1[:])
            ratio = fsb.tile([128, RC], F32, tag="ratio")
            nc.vector.tensor_mul(ratio[:], pos_ps[:], rec[:])
            nc.vector.tensor_scalar_add(ratio[:], ratio[:], 1e-8)
            lg = fsb.tile([128, RC], F32, tag="lg")
            nc.scalar.activation(lg[:], ratio[:], mybir.ActivationFunctionType.Ln)
            out_sb = fsb.tile([128, RC], F32, tag="out_sb")
            nc.vector.tensor_mul(out_sb[:], lg[:], nv_sb[:])
            nc.sync.dma_start(out[:, :], out_sb[:])

    nc.compile()
    return nc, emb_np_dt


def _get_program():
    mm1_mode = os.environ.get("CONTRASTIVE_MM1_DT", "bf16")
    if mm1_mode not in _PROGRAM_CACHE:
        _PROGRAM_CACHE[mm1_mode] = _build_program(mm1_mode)
    return _PROGRAM_CACHE[mm1_mode]


def _prepare_in_maps(embeddings, labels, emb_np_dt):
    emb = np.asarray(embeddings, dtype=np.float32)
    lab = np.asarray(labels).astype(np.int64)
    embT = np.ascontiguousarray(emb.T).astype(emb_np_dt)  # [D, N]
    classes = np.arange(C, dtype=np.int64)
    onehot = lab[:, None] == classes[None, :]  # [N, C] bool
    oh_bf16 = onehot.astype(ml_dtypes.bfloat16)
    counts = np.bincount(lab, minlength=C)
    valid = (counts[lab] - 1) > 0  # [N] bool
    negval = np.where(valid, -1.0, 0.0).astype(np.float32)
    offd = (1.0 - np.eye(128, dtype=np.float32)).astype(ml_dtypes.bfloat16)

    in_maps = []
    for i in range(NCORES):
        r0 = i * R
        in_maps.append(
            {
                "embT_cols": np.ascontiguousarray(np.roll(embT, -r0, axis=1)),
                "embT_rows": np.ascontiguousarray(embT[:, r0 : r0 + R]),
                "ohc": np.ascontiguousarray(np.roll(oh_bf16, -r0, axis=0)),
                "ohrT": np.ascontiguousarray(oh_bf16[r0 : r0 + R].T),
                # [128, RC] with row = chunk*128 + partition
                "negval": np.ascontiguousarray(negval[r0 : r0 + R].reshape(RC, 128).T),
                "offdiag": offd,
            }
        )
    return in_maps, valid


def run(embeddings, labels, trace=False, trace_cores=None):
    """Returns (mean_loss, BassKernelResults)."""
    nc, emb_np_dt = _get_program()
    in_maps, valid = _prepare_in_maps(embeddings, labels, emb_np_dt)
    kwargs = {}
    if trace:
        kwargs["trace"] = True
        if trace_cores is not None:
            kwargs["trace_cores"] = trace_cores
    res = run_bass_kernel_spmd(nc, in_maps, core_ids=list(range(NCORES)), **kwargs)
    loss_sum = 0.0
    for i in range(NCORES):
        loss_sum += float(res.results[i]["out"].astype(np.float64).sum())
    cnt = int(valid.sum())
    mean = loss_sum / cnt if cnt > 0 else 0.0
    return np.asarray(mean, dtype=np.float32), res


def kernel(embeddings, labels):
    return run(embeddings, labels)[0]


# revision 26
# speedup vs baseline: 1.2318x; 1.0403x over previous
"""Trainium2 Bass kernel for supervised contrastive loss over N=8192 rows.

Strategy (8-core SPMD, rows sharded 1024/core):
  - Per column chunk t (128 cols): simT[c, r] = emb_cols_t @ emb_rows.T via PE,
    exp(sim/T) on the scalar engine (bf16 out), diagonal zeroed by an off-diag
    mask multiply, then S_T[class, row] += onehot_colsT @ exp on PE (bf16).
    Classes partition the columns, so total_sum = sum_c S_T and positive_sum =
    sum_c S_T * onehotR.  A small per-row-chunk matmul tail produces per-row
    masked loss; the host sums partials and divides by the valid count.
  - The diagonal's chunk position is made core-invariant by rotating each
    core's column-side data (embeddings and one-hots) by its row offset.
"""

import os
import numpy as np
import ml_dtypes

import concourse.tile as tile
from concourse import bacc, mybir
from concourse.bass_utils import run_bass_kernel_spmd

N, D, C = 8192, 128, 100
NCORES = 8
R = N // NCORES  # rows per core
NT = N // 128  # column chunks of 128
RC = R // 128  # row chunks per core (8)
TEMP = 0.07
F32 = mybir.dt.float32
F32R = mybir.dt.float32r
F16 = mybir.dt.float16
BF16 = mybir.dt.bfloat16

_PROGRAM_CACHE = {}


def _build_program(mm1_mode):
    mm1_dt = {"f16": F16, "bf16": BF16, "f32r": F32R, "f32": F32}[mm1_mode]
    emb_np_dt = {
        "f16": np.float16,
        "bf16": ml_dtypes.bfloat16,
        "f32r": np.float32,
        "f32": np.float32,
    }[mm1_mode]

    nc = bacc.Bacc("TRN2", target_bir_lowering=False, debug=False, num_devices=NCORES)

    emb_dram_dt = {"f16": F16, "bf16": BF16, "f32r": F32, "f32": F32}[mm1_mode]
    embT_cols = nc.dram_tensor("embT_cols", [D, N], emb_dram_dt, kind="ExternalInput")
    embT_rows = nc.dram_tensor("embT_rows", [D, R], emb_dram_dt, kind="ExternalInput")
    ohc = nc.dram_tensor("ohc", [N, C], BF16, kind="ExternalInput")
    ohrT = nc.dram_tensor("ohrT", [C, R], BF16, kind="ExternalInput")
    negval = nc.dram_tensor("negval", [128, RC], F32, kind="ExternalInput")
    offdiag = nc.dram_tensor("offdiag", [128, 128], BF16, kind="ExternalInput")
    out = nc.dram_tensor("out", [128, RC], F32, kind="ExternalOutput")

    def bc(ap):
        # view a DRAM fp32 AP as fp32r when needed so the verifier sees
        # fp32r-typed producers for fp32r matmuls
        return ap.bitcast(F32R) if mm1_mode == "f32r" else ap

    with tile.TileContext(nc) as tc:
        with (
            tc.tile_pool(name="consts", bufs=1) as consts,
            tc.tile_pool(name="spool", bufs=1, space="PSUM") as spool,
            tc.tile_pool(name="simpool", bufs=1, space="PSUM") as simpool,
            tc.tile_pool(name="exppool", bufs=2) as exppool,
            tc.tile_pool(name="fsb", bufs=1) as fsb,
        ):
            # Resident inputs, ordered so the first chunk's dependencies land
            # first: rows, cols[0], ohc[0], then the rest streams behind
            # compute.
            # Critical-path loads first, in small pieces, so chunk 0's matmul
            # and accumulation unblock within ~1us of kernel start.
            rows_sb = consts.tile([D, R], mm1_dt, tag="rows")
            cols_sb = []
            for j in range(8):
                tcol = consts.tile([D, 1024], mm1_dt, tag=f"col{j}", name=f"cols_sb{j}")
                cols_sb.append(tcol)
            ohc_sb = consts.tile([128, NT, C], BF16, tag="ohc")
            ohc_re = ohc[:, :].rearrange("(t p) c -> p t c", p=128)

            # Embeddings on the sync queue (critical path), one-hots and small
            # constants on the gpsimd queue in parallel.
            nc.sync.dma_start(cols_sb[0][:, 0:256], bc(embT_cols[:, 0:256]))
            nc.sync.dma_start(rows_sb[:, 0:512], bc(embT_rows[:, 0:512]))
            nc.sync.dma_start(rows_sb[:, 512:R], bc(embT_rows[:, 512:R]))
            nc.sync.dma_start(cols_sb[0][:, 256:1024], bc(embT_cols[:, 256:1024]))
            for j in range(1, 8):
                nc.sync.dma_start(
                    cols_sb[j][:], bc(embT_cols[:, j * 1024 : (j + 1) * 1024])
                )
            offd_sb = consts.tile([128, 128], BF16, tag="offd")
            nc.gpsimd.dma_start(ohc_sb[:, 0:2, :], ohc_re[:, 0:2, :])
            nc.gpsimd.dma_start(offd_sb[:], offdiag[:, :])
            nc.gpsimd.dma_start(ohc_sb[:, 2:8, :], ohc_re[:, 2:8, :])
            for j in range(1, 8):
                sl = slice(j * 8, (j + 1) * 8)
                nc.gpsimd.dma_start(ohc_sb[:, sl, :], ohc_re[:, sl, :])
            ohrT_sb = consts.tile([C, R], BF16, tag="ohrT")
            nc.gpsimd.dma_start(ohrT_sb[:], ohrT[:, :])
            nv_sb = consts.tile([128, RC], F32, tag="nv")
            nc.gpsimd.dma_start(nv_sb[:], negval[:, :])

            # Preload the Ln activation table while the pipeline ramps so the
            # tail's Ln doesn't pay the table switch.
            lnpre = fsb.tile([1, 1], F32, tag="lnpre")
            nc.vector.memset(lnpre[:], 1.0)
            lnpre_out = fsb.tile([1, 1], F32, tag="lnpre_out")
            nc.scalar.activation(
                lnpre_out[:], lnpre[:], mybir.ActivationFunctionType.Ln
            )

            # S_T[class, row] accumulator over all column chunks. Split into
            # two 512-row tiles: a matmul output must stay within one PSUM bank.
            S_T = [
                spool.tile([C, 512], F32, tag=f"S{q}", name=f"S_T{q}")
                for q in range(2)
            ]

            # Column chunks are processed in alternating groups of 2 and 1 so
            # one [128, 2048] (4-bank) and one [128, 1024] (2-bank) PSUM tile
            # ping-pong, amortizing the per-ACTIVATE fixed overhead over more
            # elements. Software-pipelined: group g+1's sim+exp are issued
            # before group g's accumulation matmuls.
            # Start with a single so the first ACTIVATE fires as early as
            # possible, then alternate 2/1.
            groups = []
            t = 0
            while t < NT:
                if len(groups) % 2 == 1 and t + 1 < NT:
                    groups.append((t, t + 1))
                    t += 2
                else:
                    groups.append((t,))
                    t += 1

            exp_of_group = [None] * len(groups)

            def emit_sim_exp(g):
                chunks = groups[g]
                n = len(chunks)
                tag = "simbig" if n == 2 else "simsmall"
                sim_ps = simpool.tile([128, n * R], F32, name=f"sim{g}", tag=tag)
                for i, tt in enumerate(chunks):
                    lhsT = cols_sb[tt // 8][:, (tt % 8) * 128 : (tt % 8 + 1) * 128]
                    for h in range(2):
                        osl = slice(i * R + h * 512, i * R + (h + 1) * 512)
                        rsl = slice(h * 512, (h + 1) * 512)
                        nc.tensor.matmul(
                            sim_ps[:, osl], lhsT, rows_sb[:, rsl], start=True, stop=True
                        )
                etag = "expbig" if n == 2 else "expsmall"
                exp_sb = exppool.tile([128, n * R], BF16, name=f"exp{g}", tag=etag)
                nc.scalar.activation(
                    exp_sb[:], sim_ps[:], mybir.ActivationFunctionType.Exp,
                    scale=float(1.0 / TEMP),
                )
                for i, tt in enumerate(chunks):
                    if tt < RC:
                        # Chunk tt's columns are rows tt*128..tt*128+127 of this
                        # core: the diagonal is the main diagonal of the block.
                        blk = slice(i * R + tt * 128, i * R + (tt + 1) * 128)
                        nc.vector.tensor_mul(
                            exp_sb[:, blk], exp_sb[:, blk], offd_sb[:]
                        )
                exp_of_group[g] = exp_sb

            emit_sim_exp(0)
            emit_sim_exp(1)
            for g, chunks in enumerate(groups):
                if g + 2 < len(groups):
                    emit_sim_exp(g + 2)
                for i, tt in enumerate(chunks):
                    for q in range(2):
                        nc.tensor.matmul(
                            S_T[q][:],
                            ohc_sb[:, tt, :],
                            exp_of_group[g][:, i * R + q * 512 : i * R + (q + 1) * 512],
                            start=(tt == 0),
                            stop=(tt == NT - 1),
                        )

            # Tail: per-row totals via small matmuls so everything stays in a
            # [128, RC] layout (row = chunk*128 + partition).
            S_sb = fsb.tile([C, R], BF16, tag="S_sb")
            for q in range(2):
                nc.vector.tensor_copy(S_sb[:, q * 512 : (q + 1) * 512], S_T[q][:])
            posS = fsb.tile([C, R], BF16, tag="posS")
            nc.vector.tensor_mul(posS[:], S_sb[:], ohrT_sb[:])

            # Reuse the (released) sim PSUM slots for the tiny tail outputs —
            # all 8 banks are otherwise committed.
            tot_ps = simpool.tile([128, RC], F32, tag="simbig", name="tot_ps")
            pos_ps = simpool.tile([128, RC], F32, tag="simsmall", name="pos_ps")
            ones_bf = consts.tile([C, 1], BF16, tag="ones_bf")
            nc.vector.memset(ones_bf[:], 1.0)
            for j in range(RC):
                sl = slice(j * 128, (j + 1) * 128)
                nc.tensor.matmul(
                    tot_ps[:, j : j + 1], S_sb[:, sl], ones_bf[:], start=True, stop=True
                )
                nc.tensor.matmul(
                    pos_ps[:, j : j + 1], posS[:, sl], ones_bf[:], start=True, stop=True
                )
            t1 = fsb.tile([128, RC], F32, tag="t1")
            nc.vector.tensor_scalar_add(t1[:], tot_ps[:], 1e-8)
            rec = fsb.tile([128, RC], F32, tag="rec")
            nc.vector.reciprocal(rec[:], t1[:])
            ratio = fsb.tile([128, RC], F32, tag="ratio")
            nc.vector.tensor_mul(ratio[:], pos_ps[:], rec[:])
            nc.vector.tensor_scalar_add(ratio[:], ratio[:], 1e-8)
            lg = fsb.tile([128, RC], F32, tag="lg")
            nc.scalar.activation(lg[:], ratio[:], mybir.ActivationFunctionType.Ln)
            out_sb = fsb.tile([128, RC], F32, tag="out_sb")
            nc.vector.tensor_mul(out_sb[:], lg[:], nv_sb[:])
            nc.sync.dma_start(out[:, :], out_sb[:])

    nc.compile()
    return nc, emb_np_dt


def _get_program():
    mm1_mode = os.environ.get("CONTRASTIVE_MM1_DT", "bf16")
    if mm1_mode not in _PROGRAM_CACHE:
        _PROGRAM_CACHE[mm1_mode] = _build_program(mm1_mode)
    return _PROGRAM_CACHE[mm1_mode]


def _prepare_in_maps(embeddings, labels, emb_np_dt):
    emb = np.asarray(embeddings, dtype=np.float32)
    lab = np.asarray(labels).astype(np.int64)
    embT = np.ascontiguousarray(emb.T).astype(emb_np_dt)  # [D, N]
    classes = np.arange(C, dtype=np.int64)
    onehot = lab[:, None] == classes[None, :]  # [N, C] bool
    oh_bf16 = onehot.astype(ml_dtypes.bfloat16)
    counts = np.bincount(lab, minlength=C)
    valid = (counts[lab] - 1) > 0  # [N] bool
    negval = np.where(valid, -1.0, 0.0).astype(np.float32)
    offd = (1.0 - np.eye(128, dtype=np.float32)).astype(ml_dtypes.bfloat16)

    in_maps = []
    for i in range(NCORES):
        r0 = i * R
        in_maps.append(
            {
                "embT_cols": np.ascontiguousarray(np.roll(embT, -r0, axis=1)),
                "embT_rows": np.ascontiguousarray(embT[:, r0 : r0 + R]),
                "ohc": np.ascontiguousarray(np.roll(oh_bf16, -r0, axis=0)),
                "ohrT": np.ascontiguousarray(oh_bf16[r0 : r0 + R].T),
                # [128, RC] with row = chunk*128 + partition
                "negval": np.ascontiguousarray(negval[r0 : r0 + R].reshape(RC, 128).T),
                "offdiag": offd,
            }
        )
    return in_maps, valid


def run(embeddings, labels, trace=False, trace_cores=None):
    """Returns (mean_loss, BassKernelResults)."""
    nc, emb_np_dt = _get_program()
    in_maps, valid = _prepare_in_maps(embeddings, labels, emb_np_dt)
    kwargs = {}
    if trace:
        kwargs["trace"] = True
        if trace_cores is not None:
            kwargs["trace_cores"] = trace_cores
    res = run_bass_kernel_spmd(nc, in_maps, core_ids=list(range(NCORES)), **kwargs)
    loss_sum = 0.0
    for i in range(NCORES):
        loss_sum += float(res.results[i]["out"].astype(np.float64).sum())
    cnt = int(valid.sum())
    mean = loss_sum / cnt if cnt > 0 else 0.0
    return np.asarray(mean, dtype=np.float32), res


def kernel(embeddings, labels):
    return run(embeddings, labels)[0]
